# revision 25
# baseline (speedup 1.0000x reference)
"""Causal self-attention (B=4, T=2048, C=1024, H=16) on 8 NeuronCores.

Sharding: data-parallel over batch (4) x tensor-parallel over heads (2 groups
of 8 heads) = 8 cores. Each core computes QKV for its 8 heads, causal
flash-style attention, and a partial output projection (row-parallel).
Host sums the two partial projections per batch and adds b_proj.

All matmul operands are stored fp16 (the PE multiplies at ~fp22 internally,
so fp16's 11-bit mantissa matches fp32r precision while halving memory and
enabling hardware DMA-transpose + fast weight loads). All accumulation is
fp32 in PSUM.

Per-core device kernel (Bass/Tile):
  phase 1: x^T loaded via hardware DMA-transpose (fp16); q^T,k^T [ch,T] and
           v [T,ch] (65-col blocks with a ones column that makes the PV
           matmul emit softmax denominators) via fp16 matmuls; qk bias fused
           into the PSUM->SBUF copy, v bias as a K=1 matmul.
  phase 2: per (head, 512-wide tq chunk): scores^T = k^T.T @ q^T in PSUM,
           exp on ACT (scale=1/8) -> P^T fp16, causal handled by restricting
           diagonal-block columns + affine_select zero-fill, PV accumulate
           y^T[65,512] where row 64 = softmax denominator l. Normalization:
           r=1/l broadcast to [64,512] with a K=1 PE matmul, applied on DVE.
           Phase-1 chunks for later tq are interleaved into this stream so
           the ACT-bound exp work overlaps PE-bound qkv matmuls.
  phase 3 (interleaved per tq chunk): out = y^T.T @ w_proj, DMA out (fp32).
"""

from contextlib import nullcontext

import numpy as np

import concourse.bass as bass
import concourse.mybir as mybir
from concourse import bacc
from concourse.tile import TileContext
from concourse.bass_utils import run_bass_kernel_spmd

B, T, C, H, D = 4, 2048, 1024, 16, 64
CQ = 512          # q (or k or v) channels per core = 8 heads * 64
HPC = 8           # heads per core
F32 = mybir.dt.float32
F16 = mybir.dt.float16
Exp = mybir.ActivationFunctionType.Exp
is_ge = mybir.AluOpType.is_ge

TCH = 512         # phase-1 T-chunk
NCH = T // TCH    # 8 chunks
VSTR = HPC * (D + 1)   # 520: v_ext per-T-block stride (8 heads x 65)


def build_nc(loop_n=1):
    """loop_n > 1 wraps the whole kernel in a device-side repeat loop
    (benchmarking only -- output is identical every iteration)."""
    nc = bacc.Bacc("TRN2", target_bir_lowering=False, debug=False, num_devices=8)

    x = nc.dram_tensor("x", [T, C], F16, kind="ExternalInput")
    w_qk = nc.dram_tensor("w_qk", [C, 2 * CQ], F16, kind="ExternalInput")
    w_v = nc.dram_tensor("w_v", [C, CQ], F16, kind="ExternalInput")
    b_qk = nc.dram_tensor("b_qk", [1, 2 * CQ], F32, kind="ExternalInput")
    b_vz = nc.dram_tensor("b_vz", [128, CQ], F16, kind="ExternalInput")
    onesz = nc.dram_tensor("onesz", [128, 128], F16, kind="ExternalInput")
    w_pj = nc.dram_tensor("w_pj", [CQ, C], F16, kind="ExternalInput")
    out = nc.dram_tensor("out", [T, C], F32, kind="ExternalOutput")

    with TileContext(nc) as tc:
        with (
            tc.tile_pool(name="const", bufs=1) as pc,
            tc.tile_pool(name="persist", bufs=1) as pp,
            tc.tile_pool(name="work", bufs=2) as pw,
            tc.tile_pool(name="psum", bufs=2, space="PSUM") as ps,
            tc.For_i(0, loop_n, 1) if loop_n > 1 else nullcontext(),
        ):
            # ---- constants ----
            w_qk_sb = pc.tile([128, 8 * 1024], F16, name="w_qk_sb")
            for kk in range(8):
                nc.sync.dma_start(
                    out=w_qk_sb[:, kk * 1024:(kk + 1) * 1024],
                    in_=w_qk[kk * 128:(kk + 1) * 128, :],
                )
            # per-channel qk bias as [128, 8] (partition = ch within tile)
            b_qk2 = pc.tile([128, 8], F32, name="b_qk2")
            nc.sync.dma_start(
                out=b_qk2[:], in_=b_qk[0, :].rearrange("(m p) -> p m", p=128)
            )
            b_vz_sb = pc.tile([128, CQ], F16, name="b_vz_sb")
            nc.sync.dma_start(out=b_vz_sb[:], in_=b_vz[:])
            onesz_sb = pc.tile([128, 128], F16, name="onesz_sb")
            nc.sync.dma_start(out=onesz_sb[:], in_=onesz[:])
            w_v_sb = pc.tile([128, 8 * 512], F16, name="w_v_sb")
            for kk in range(8):
                nc.sync.dma_start(
                    out=w_v_sb[:, kk * 512:(kk + 1) * 512],
                    in_=w_v[kk * 128:(kk + 1) * 128, :],
                )

            # ---- persistent activations ----
            qT = [pp.tile([128, T], F16, name=f"qT{m}") for m in range(4)]
            # per-head k^T, zero-padded to K=128 so the scores matmul streams
            # the full qT tile at full SBUF bandwidth (the zero half
            # multiplies the sibling head's rows away)
            kZ = [pp.tile([128, T], F16, name=f"kZ{i}") for i in range(HPC)]
            for i in range(HPC):
                z0 = 64 * (1 - i % 2)
                nc.vector.memset(kZ[i][z0:z0 + 64, :], 0.0)
            yT = [pp.tile([128, T], F16, name=f"yT{m}") for m in range(4)]
            v_ext = pp.tile([128, (T // 128) * VSTR], F16, name="v_ext")
            r_t = pp.tile([128, 512], F16, name="r_t")
            nc.vector.memset(r_t[:], 0.0)
            v_ones = v_ext[:].rearrange(
                "p (t i d) -> p t i d", i=HPC, d=D + 1
            )[:, :, :, D:D + 1]
            nc.gpsimd.memset(v_ones, 1.0)

            def phase1_chunk_steps(ct):
                """Yield emission closures for one phase-1 chunk, so chunks
                can be interleaved into the attention stream."""
                T0 = ct * TCH
                xT_c = pw.tile([128, 8 * TCH], F16, name="xT_c", tag="xT_c", bufs=2)

                def xload():
                    # x^T tiles straight from DRAM via hardware DMA transpose
                    for kk in range(8):
                        nc.sync.dma_start_transpose(
                            xT_c[:, kk * TCH:(kk + 1) * TCH],
                            x[T0:T0 + TCH, kk * 128:(kk + 1) * 128],
                        )

                def qk(m0):
                    # q,k: out^T layout [ch, T-chunk]; bias fused into copy
                    for m in range(m0, m0 + 4):
                        qk_ps = ps.tile([128, TCH], F32, name="qk_ps", tag="mm", bufs=2)
                        for kk in range(8):
                            nc.tensor.matmul(
                                qk_ps[:],
                                w_qk_sb[:, kk * 1024 + m * 128:kk * 1024 + (m + 1) * 128],
                                xT_c[:, kk * TCH:(kk + 1) * TCH],
                                start=(kk == 0),
                                stop=(kk == 7),
                            )
                        if m < 4:
                            nc.vector.tensor_scalar_add(
                                qT[m][:, T0:T0 + TCH], qk_ps[:], b_qk2[:, m:m + 1]
                            )
                        else:
                            for half in range(2):
                                ih = 2 * (m - 4) + half
                                rows = slice(64 * half, 64 * half + 64)
                                nc.vector.tensor_scalar_add(
                                    kZ[ih][rows, T0:T0 + TCH],
                                    qk_ps[rows, :],
                                    b_qk2[rows, m:m + 1],
                                )

                def vpart(tt):
                    # v: natural layout [T-block, ch], interleaved into v_ext
                    v_ps = ps.tile([128, CQ], F32, name="v_ps", tag="mm", bufs=2)
                    for kk in range(8):
                        nc.tensor.matmul(
                            v_ps[:],
                            xT_c[:, kk * TCH + tt * 128:kk * TCH + (tt + 1) * 128],
                            w_v_sb[:, kk * 512:(kk + 1) * 512],
                            start=(kk == 0),
                            stop=False,
                        )
                    nc.tensor.matmul(
                        v_ps[:],
                        onesz_sb[:, 0:128],
                        b_vz_sb[:],
                        start=False,
                        stop=True,
                    )
                    tb = ct * (TCH // 128) + tt
                    dst = v_ext[:, tb * VSTR:(tb + 1) * VSTR].rearrange(
                        "p (i d) -> p i d", d=D + 1
                    )[:, :, 0:D]
                    src = v_ps[:].rearrange("p (i d) -> p i d", d=D)
                    nc.vector.tensor_copy(dst, src)

                yield xload
                yield lambda: qk(0)
                yield lambda: qk(4)
                for t0 in range(0, TCH // 128, 2):
                    yield lambda t0=t0: (vpart(t0), vpart(t0 + 1))

            def phase1_chunk(ct):
                for step in phase1_chunk_steps(ct):
                    step()

            def attention(c, i):
                """Emit scores+exp / PV as a 2-stage software pipeline (PV one
                group behind) so the in-order PE stream never waits on ACT.
                Returns (y_ps, m, p0, c) for deferred normalization."""
                m = i // 2
                p0 = 64 * (i % 2)
                nblk = 4 * c + 4
                y_ps = ps.tile([D + 1, 512], F32, name="y_ps", tag="psy", bufs=2)

                def vslice(tkb):
                    return v_ext[
                        :, tkb * VSTR + i * (D + 1):tkb * VSTR + (i + 1) * (D + 1)
                    ]

                pending = []  # [(P tile, [(tkb, out_col0, p_col0, w), ...])]

                def flush(depth=0):
                    # emit PV for queued groups, keeping `depth` in flight so
                    # the PE stream stays 2 exp-latencies ahead
                    while len(pending) > depth:
                        P, items = pending.pop(0)
                        for tkb, oc0, pc0, w in items:
                            nc.tensor.matmul(
                                y_ps[:, oc0:oc0 + w],
                                vslice(tkb),
                                P[:, pc0:pc0 + w],
                                start=(tkb == 0),
                                stop=(tkb == nblk - 1),
                                skip_group_check=True,
                            )

                def group(items):
                    """One psum tile + one exp over several blocks.
                    items: (tkb, out_col0, p_col0, width, straddler)."""
                    nonlocal pending
                    total = items[-1][2] + items[-1][3]
                    s_g = ps.tile([128, 1024], F32, name="s_g", tag="mm", bufs=2)
                    P_g = pw.tile([128, 1024], F16, name="P_g", tag="P_t", bufs=8)
                    for tkb, oc0, pc0, w, _ in items:
                        nc.tensor.matmul(
                            s_g[:, pc0:pc0 + w],
                            kZ[i][:, tkb * 128:(tkb + 1) * 128],
                            qT[m][:, c * 512 + oc0:(c + 1) * 512],
                            start=True,
                            stop=True,
                        )
                    nc.scalar.activation(
                        P_g[:, 0:total], s_g[:, 0:total], Exp, scale=0.125)
                    for tkb, oc0, pc0, w, straddler in items:
                        if straddler:
                            # keep where (piece-local y) >= x
                            nc.gpsimd.affine_select(
                                out=P_g[:, pc0:pc0 + w],
                                in_=P_g[:, pc0:pc0 + w],
                                compare_op=is_ge,
                                fill=0.0,
                                base=0,
                                pattern=[[1, w]],
                                channel_multiplier=-1,
                            )
                    flush(depth=2)
                    pending.append((P_g, [it[:4] for it in items]))

                # full (below-diagonal) blocks in pairs; diagonal straddlers
                # packed j0+j1 and j2+j3 to amortize ACT fixed cost
                for pair in range(2 * c):
                    group([(2 * pair, 0, 0, 512, False),
                           (2 * pair + 1, 0, 512, 512, False)])
                group([(4 * c, 0, 0, 512, True),
                       (4 * c + 1, 128, 512, 384, True)])
                group([(4 * c + 2, 256, 0, 256, True),
                       (4 * c + 3, 384, 256, 128, True)])
                flush()
                return (y_ps, m, p0, c)

            def normalize(pend):
                y_ps, m, p0, c = pend
                with nc.allow_low_precision(reason="fp16 matches PE fp22 input precision"):
                    nc.vector.reciprocal(r_t[0:1, :], y_ps[D:D + 1, :])
                R_ps = ps.tile([64, 512], F32, name="R_ps", tag="pssm", bufs=2)
                nc.tensor.matmul(
                    R_ps[:],
                    onesz_sb[:, 0:64],
                    r_t[:],
                    start=True,
                    stop=True,
                )
                R_sb = pw.tile([64, 512], F32, name="R_sb", tag="R_sb", bufs=2)
                nc.vector.tensor_copy(R_sb[:], R_ps[:])
                with nc.allow_low_precision(reason="fp16 matches PE fp22 input precision"):
                    nc.vector.tensor_mul(
                        yT[m][p0:p0 + 64, c * 512:(c + 1) * 512],
                        y_ps[0:D, :],
                        R_sb[:],
                    )

            def proj(mt):
                o_t = pw.tile([128, C], F32, name="o_t", tag="o_t", bufs=2)
                for nn in range(2):
                    pj_ps = ps.tile([128, 512], F32, name="pj_ps", tag="mm", bufs=2)
                    for kk in range(4):
                        nc.tensor.matmul(
                            pj_ps[:],
                            yT[kk][:, mt * 128:(mt + 1) * 128],
                            w_pj_sb[:, kk * 1024 + nn * 512:kk * 1024 + (nn + 1) * 512],
                            start=(kk == 0),
                            stop=(kk == 3),
                        )
                    nc.vector.tensor_copy(o_t[:, nn * 512:(nn + 1) * 512], pj_ps[:])
                nc.gpsimd.dma_start(out=out[mt * 128:(mt + 1) * 128, :], in_=o_t[:])

            # ---- emission: chunks 0-1 up front (attention c=0 needs them),
            # remaining phase-1 chunks interleaved into the attention stream
            # so ACT-bound exp work overlaps PE-bound qkv matmuls.
            phase1_chunk(0)
            phase1_chunk(1)

            w_pj_sb = pc.tile([128, 4 * 1024], F16, name="w_pj_sb")
            for kk in range(4):
                nc.sync.dma_start(
                    out=w_pj_sb[:, kk * 1024:(kk + 1) * 1024],
                    in_=w_pj[kk * 128:(kk + 1) * 128, :],
                )

            p1_queue = []
            for ct in range(2, NCH):
                p1_queue.extend(phase1_chunk_steps(ct))

            # normalization of head i emitted after head i+1's blocks so the
            # PE never waits on the DVE reciprocal chain
            pend = None
            proj_q = []
            for c in range(T // 512):
                for i in range(HPC):
                    nxt = attention(c, i)
                    if pend is not None:
                        normalize(pend)
                    pend = nxt
                    # later phase-1 chunks emitted during attention chunk c
                    # (needed by attention chunk c+1)
                    if p1_queue and c < 3:
                        p1_queue.pop(0)()
                    # previous chunk's projection spread over this chunk's heads
                    if proj_q:
                        proj_q.pop(0)()
                normalize(pend)
                pend = None
                proj_q = [lambda mt=mt: proj(mt) for mt in range(4 * c, 4 * c + 4)]
            for f in proj_q:
                f()

    nc.compile()
    return nc


_NC = None


def _get_nc():
    global _NC
    if _NC is None:
        _NC = build_nc()
    return _NC


def make_in_maps(x, w_attn, b_attn, w_proj):
    x = np.asarray(x, dtype=np.float32)
    w_attn = np.asarray(w_attn, dtype=np.float32)
    b_attn = np.asarray(b_attn, dtype=np.float32)
    w_proj = np.asarray(w_proj, dtype=np.float32)
    in_maps = []
    for core in range(8):
        b, g = divmod(core, 2)
        s = g * CQ
        in_maps.append({
            "x": np.ascontiguousarray(x[b]).astype(np.float16),
            "w_qk": np.ascontiguousarray(
                np.concatenate([w_attn[:, s:s + CQ], w_attn[:, C + s:C + s + CQ]], axis=1)
            ).astype(np.float16),
            "w_v": np.ascontiguousarray(
                w_attn[:, 2 * C + s:2 * C + s + CQ]).astype(np.float16),
            "b_qk": np.concatenate(
                [b_attn[s:s + CQ], b_attn[C + s:C + s + CQ]]
            ).reshape(1, 2 * CQ).astype(np.float32),
            "b_vz": np.concatenate([
                b_attn[2 * C + s:2 * C + s + CQ].reshape(1, CQ),
                np.zeros((127, CQ), np.float32)]).astype(np.float16),
            "onesz": np.concatenate([
                np.ones((1, 128), np.float32),
                np.zeros((127, 128), np.float32)]).astype(np.float16),
            "w_pj": np.ascontiguousarray(w_proj[s:s + CQ, :]).astype(np.float16),
        })
    return in_maps


def kernel(x, w_attn, b_attn, w_proj, b_proj):
    nc = _get_nc()
    in_maps = make_in_maps(x, w_attn, b_attn, w_proj)
    res = run_bass_kernel_spmd(nc, in_maps, list(range(8)))
    b_proj = np.asarray(b_proj, dtype=np.float32)
    out = np.empty((B, T, C), dtype=np.float32)
    for b in range(B):
        out[b] = res.results[2 * b]["out"] + res.results[2 * b + 1]["out"] + b_proj
    return out
